# revision 42
# baseline (speedup 1.0000x reference)
"""Block sliding-window attention on 8 TRN2 NeuronCores.

Sharding: sequence-parallel, no collectives. 8 shards = (batch b in {0,1}) x
(quarter s in 0..3); each core owns 2048 consecutive tokens of one batch plus
a 256-token K/V halo from the previous quarter (zeros + -inf gate for the
first quarter).

All four projection matmuls run as fp8-e4m3 DoubleRow (2 contraction rows per
partition, 2x PE column rate) with a 3-term residual split for bf16-level
accuracy:  x@W ~= xh@Wh + xh@Wl + xl@Wh,  where xh = fp8(s*x) and
xl = fp8(s*x - xh) share one scale per tensor, so all three terms accumulate
in a single PSUM group (cost 12 col-cycles per 2048-deep column vs 16 bf16).

Per-core pipeline:
  P1: Q = x@Wq with RoPE fused on the PSUM path (ACT scaled copy -> bf16,
      partition-swap SBUF DMA for rotate-half, 3 DVE bf16 muls/adds with
      bf16 cos / pre-signed sin), roped Q kept resident in SBUF; roped K
      stored chunk-major to DRAM; V natural to DRAM.
  P2: per 256-token chunk and head: S^T = K Q^T per 128-key block into one
      [128,1024] PSUM tile, exp on ACT (scale 1/sqrt(128), -1e30 bias gates
      the no-previous case), 0/1 triangular mask multiply on DVE,
      denominator via all-ones matmul, O^T = V^T P^T, reciprocal on DVE,
      normalize on GpSimd(Pool), store O^T token-tile-major bf16.
  P3: out = O @ Wo via the same fp8 DoubleRow split (O quantized on the fly:
      ACT scaled copy for hi, fused DVE scalar_tensor_tensor for lo).
"""
import sys

try:
    import concourse  # noqa: F401
except ImportError:
    sys.path.insert(0, '/opt/trn_rl_repo')

import ml_dtypes
import numpy as np

import concourse.bacc as bacc
import concourse.mybir as mybir
import concourse.tile as tile
from concourse.bass_utils import run_bass_kernel_spmd

f32 = mybir.dt.float32
bf16 = mybir.dt.bfloat16
f8 = mybir.dt.float8e4
AF = mybir.ActivationFunctionType
ALU = mybir.AluOpType
DR = mybir.MatmulPerfMode.DoubleRow

DIMS = 2048
HEADS = 16
HD = 128           # head dim
WIN = 256          # window / chunk
B, S = 2, 8192
NSH = 4            # seq shards per batch
THETA = 10000.0
ISQ = float(1.0 / np.sqrt(HD))
SX = 16.0          # fp8 scale for activations (x, O)
SW = 1024.0        # fp8 scale for weights
OSC = float(1.0 / (SX * SW))
E4M3 = ml_dtypes.float8_e4m3


def tok_tiles(n):
    out, a = [], 0
    while a < n:
        w = min(512, n - a)
        out.append((a, w))
        a += w
    return out


def build(nc, T, phases=(1, 2, 3)):
    """Emit the per-core program. T = local tokens (multiple of 512)."""
    TH = T + WIN                      # with halo
    NC_ = T // WIN                    # chunks
    NTB = TH // 128                   # 128-token blocks incl halo
    NTT = T // 128                    # own 128-token blocks

    XHL = nc.dram_tensor("XHL", [128, 16, 2, TH], f8, kind="ExternalInput")
    WQHL = nc.dram_tensor("WQHL", [16, 128, 16, 2, 128], f8, kind="ExternalInput")
    WKHL = nc.dram_tensor("WKHL", [16, 128, 16, 2, 128], f8, kind="ExternalInput")
    WVHL = nc.dram_tensor("WVHL", [4, 128, 16, 2, 512], f8, kind="ExternalInput")
    WOHL = nc.dram_tensor("WOHL", [128, 16, 2, DIMS], f8, kind="ExternalInput")
    COSS = nc.dram_tensor("COSS", [128, TH], bf16, kind="ExternalInput")
    SINS = nc.dram_tensor("SINS", [128, TH], bf16, kind="ExternalInput")
    TRI23 = nc.dram_tensor("TRI23", [128, 2 * WIN], bf16, kind="ExternalInput")
    PGATE = nc.dram_tensor("PGATE", [128, 1], f32, kind="ExternalInput")
    ONESM = nc.dram_tensor("ONESM", [128, 128], bf16, kind="ExternalInput")
    OUT = nc.dram_tensor("OUT", [T, DIMS], f32, kind="ExternalOutput")

    KTS = nc.dram_tensor("KTS", [128, NC_ + 1, HEADS, WIN], bf16)  # chunk-major roped K^T
    VS = nc.dram_tensor("VS", [TH, DIMS], bf16)                    # V natural
    OTS = nc.dram_tensor("OTS", [128, NTT, HEADS, 128], bf16)      # tt-major normalized O^T

    with tile.TileContext(nc) as tc:
        with tc.tile_pool(name="cst", bufs=1) as cst, \
             tc.tile_pool(name="qres", bufs=1) as qres:
            tri23 = cst.tile([128, 2 * WIN], bf16)
            pgate = cst.tile([128, 1], f32)
            onesm = cst.tile([128, 128], bf16)
            nc.scalar.dma_start(tri23[:], TRI23[:])
            nc.scalar.dma_start(pgate[:], PGATE[:])
            nc.scalar.dma_start(onesm[:], ONESM[:])
            qts = qres.tile([128, HEADS, T], bf16)   # roped Q^T, SBUF-resident

            # ---------------- P1: projections + RoPE ----------------
            if 1 in phases:
              with tc.tile_pool(name="p1x", bufs=1) as p1x, \
                 tc.tile_pool(name="trig", bufs=1) as trig, \
                 tc.tile_pool(name="wp", bufs=2) as wp, \
                 tc.tile_pool(name="wvp", bufs=2) as wvp, \
                 tc.tile_pool(name="vbp", bufs=3) as vbp, \
                 tc.tile_pool(name="rp", bufs=3) as rp, \
                 tc.tile_pool(name="pp", bufs=6, space="PSUM") as pp, \
                 tc.tile_pool(name="ppv", bufs=2, space="PSUM") as ppv:
                xhl = p1x.tile([128, 16, 2, TH], f8)
                wt0 = wp.tile([128, 16, 2, 128], f8, name="wt")
                with tc.high_priority():
                    nc.sync.dma_start(wt0[:], WQHL[0])
                for hl in (0, 1):
                    for g in range(0, 16, 2):
                        nc.sync.dma_start(xhl[:, g:g + 2, hl],
                                          XHL[:, g:g + 2, hl])
                coss = trig.tile([128, TH], bf16)
                sins = trig.tile([128, TH], bf16)
                nc.scalar.dma_start(coss[:], COSS[:])
                nc.scalar.dma_start(sins[:], SINS[:])

                # Q (toff=WIN, RoPE pos=WIN+a, dest SBUF qts) and
                # K (toff=0, RoPE pos=a, dest DRAM KTS chunk-major)
                for W_, toff, tlen, is_q in ((WQHL, WIN, T, True),
                                             (WKHL, 0, TH, False)):
                    for ob in range(HEADS):
                        if is_q and ob == 0:
                            wt = wt0
                        else:
                            wt = wp.tile([128, 16, 2, 128], f8, name="wt")
                            nc.sync.dma_start(wt[:], W_[ob])
                        for a, w in tok_tiles(tlen):
                            ps = pp.tile([128, 512], f32, name="ps")
                            tsl = slice(toff + a, toff + a + w)
                            for kg in range(8):
                                nc.tensor.matmul(
                                    ps[:, :w], wt[:, 2 * kg:2 * kg + 2, 1],
                                    xhl[:, 2 * kg:2 * kg + 2, 0, tsl],
                                    start=(kg == 0), stop=False, perf_mode=DR)
                            for g in range(16):
                                nc.tensor.matmul(
                                    ps[:, :w], wt[:, g, :],
                                    xhl[:, g, :, tsl],
                                    start=False, stop=(g == 15), perf_mode=DR)
                            raw = rp.tile([128, 512], bf16, name="raw")
                            nc.scalar.activation(raw[:, :w], ps[:, :w],
                                                 AF.Copy, scale=OSC)
                            rot = rp.tile([128, 512], bf16, name="rot")
                            nc.sync.dma_start(rot[0:64, :w], raw[64:128, :w])
                            nc.sync.dma_start(rot[64:128, :w], raw[0:64, :w])
                            pos = toff + a
                            t1 = rp.tile([128, 512], bf16, name="t1")
                            nc.vector.tensor_mul(t1[:, :w], rot[:, :w],
                                                 sins[:, pos:pos + w])
                            t2 = rp.tile([128, 512], bf16, name="t2")
                            nc.vector.tensor_mul(t2[:, :w], raw[:, :w],
                                                 coss[:, pos:pos + w])
                            if is_q:
                                nc.vector.tensor_add(
                                    qts[:, ob, a:a + w], t2[:, :w], t1[:, :w])
                            else:
                                rop = rp.tile([128, 512], bf16, name="rop")
                                nc.vector.tensor_add(rop[:, :w], t2[:, :w],
                                                     t1[:, :w])
                                c0 = a // WIN
                                nw = w // WIN
                                nc.sync.dma_start(
                                    KTS[:, c0:c0 + nw, ob], rop[:, :w])

                # V natural: lhsT = x block [128in(x2), 128tok], rhs = Wv rows
                for og in range(4):
                    wv = wvp.tile([128, 16, 2, 512], f8, name="wv")
                    nc.sync.dma_start(wv[:], WVHL[og])
                    for tb in range(NTB):
                        ps = ppv.tile([128, 512], f32, name="psv")
                        tsl = slice(tb * 128, (tb + 1) * 128)
                        for kg in range(8):
                            nc.tensor.matmul(
                                ps[:], xhl[:, 2 * kg:2 * kg + 2, 0, tsl],
                                wv[:, 2 * kg:2 * kg + 2, 1],
                                start=(kg == 0), stop=False, perf_mode=DR)
                            nc.tensor.matmul(
                                ps[:], xhl[:, 2 * kg, :, tsl],
                                wv[:, 2 * kg, :],
                                start=False, stop=False, perf_mode=DR)
                            nc.tensor.matmul(
                                ps[:], xhl[:, 2 * kg + 1, :, tsl],
                                wv[:, 2 * kg + 1, :],
                                start=False, stop=(kg == 7), perf_mode=DR)
                        vb = vbp.tile([128, 512], bf16, name="vb")
                        nc.vector.tensor_scalar_mul(vb[:], ps[:], OSC)
                        nc.sync.dma_start(
                            VS[tb * 128:(tb + 1) * 128,
                               og * 512:(og + 1) * 512], vb[:])

            # ---------------- P3 weight prefetch + P2: attention ----------------
            with tc.tile_pool(name="wo3", bufs=1) as wo3:
              wo = wo3.tile([128, 16, 2, DIMS], f8)

              if 2 in phases:
                with tc.tile_pool(name="ktp", bufs=3) as ktp, \
                   tc.tile_pool(name="qk", bufs=4) as qk, \
                   tc.tile_pool(name="pbp", bufs=3) as pbp, \
                   tc.tile_pool(name="obp", bufs=4) as obp, \
                   tc.tile_pool(name="ps_s", bufs=3, space="PSUM") as ps_s, \
                   tc.tile_pool(name="ps_dpo", bufs=2, space="PSUM") as ps_dpo:
                    def kload(c):
                        kt = ktp.tile([128, HEADS, WIN], bf16, name="kt")
                        nc.sync.dma_start(kt[:], KTS[:, c])
                        return kt

                    def vload(c):
                        v = qk.tile([128, 2, DIMS], bf16, name="v")
                        nc.sync.dma_start(
                            v[:], VS[c * WIN:(c + 1) * WIN]
                            .rearrange("(tb p) c -> p tb c", p=128))
                        return v

                    kt_t = [kload(0), kload(1)]
                    v_t = [vload(0), vload(1)]

                    # S^T layout [keys, queries], 896 cols per head:
                    # kb0 [0:256], kb1 [256:512], kb2 [512:768] (left 128
                    # triangular), kb3-right [768:896] (triangular; the
                    # fully-masked kb3 x q<128 quadrant is never computed).
                    NSC = 3 * WIN + 128
                    def emit_scores(c, h, kts):
                        ps = ps_s.tile([128, NSC], f32, name="ps")
                        for kb in range(3):
                            nc.tensor.matmul(
                                ps[:, kb * WIN:(kb + 1) * WIN],
                                kts[kb // 2][:, h, (kb % 2) * 128:(kb % 2) * 128 + 128],
                                qts[:, h, c * WIN:(c + 1) * WIN],
                                start=True, stop=True)
                        nc.tensor.matmul(
                            ps[:, 3 * WIN:], kts[1][:, h, 128:256],
                            qts[:, h, c * WIN + 128:(c + 1) * WIN],
                            start=True, stop=True)
                        return ps

                    def emit_tail(c, h, ps, vs):
                        pb = pbp.tile([128, NSC], bf16, name="pb")
                        if c == 0:
                            nc.scalar.activation(pb[:, 0:2 * WIN], ps[:, 0:2 * WIN],
                                                 AF.Exp, bias=pgate[:], scale=ISQ)
                            nc.scalar.activation(pb[:, 2 * WIN:], ps[:, 2 * WIN:],
                                                 AF.Exp, scale=ISQ)
                        else:
                            nc.scalar.activation(pb[:], ps[:], AF.Exp, scale=ISQ)
                        tri1 = tri23[:, 0:128]
                        nc.gpsimd.tensor_mul(pb[:, 512:640], pb[:, 512:640], tri1)
                        nc.gpsimd.tensor_mul(pb[:, 768:896], pb[:, 768:896], tri1)
                        pdo = ps_dpo.tile([128, 2 * WIN], f32, name="pdo")
                        pd = pdo[:, 0:WIN]
                        po = pdo[:, WIN:2 * WIN]
                        for kb in range(3):
                            nc.tensor.matmul(
                                pd, onesm[:], pb[:, kb * WIN:(kb + 1) * WIN],
                                start=(kb == 0), stop=False)
                        nc.tensor.matmul(
                            pdo[:, 128:WIN], onesm[:], pb[:, 3 * WIN:],
                            start=False, stop=True, skip_group_check=True)
                        for kb in range(3):
                            nc.tensor.matmul(
                                po, vs[kb // 2][:, kb % 2, h * 128:(h + 1) * 128],
                                pb[:, kb * WIN:(kb + 1) * WIN],
                                start=(kb == 0), stop=False)
                        nc.tensor.matmul(
                            pdo[:, WIN + 128:2 * WIN],
                            vs[1][:, 1, h * 128:(h + 1) * 128],
                            pb[:, 3 * WIN:],
                            start=False, stop=True, skip_group_check=True)
                        rb = obp.tile([128, WIN], f32, name="rb")
                        with nc.allow_low_precision("softmax denominator"):
                            nc.vector.reciprocal(rb[:], pd)
                        ot = obp.tile([128, WIN], bf16, name="ot")
                        nc.vector.tensor_mul(ot[:], po, rb[:])
                        nc.sync.dma_start(OTS[:, 2 * c:2 * c + 2, h], ot[:])

                    pending = []
                    for c in range(NC_):
                        if c + 2 <= NC_:
                            kt_t.append(kload(c + 2))
                            v_t.append(vload(c + 2))
                        if c >= 1:
                            per = -(-16 // max(1, NC_ - 1))
                            for g in range(per * (c - 1),
                                           min(16, per * c)):
                                nc.sync.dma_start(wo[:, g], WOHL[:, g])
                        kts = (kt_t[c], kt_t[c + 1])
                        vs = (v_t[c], v_t[c + 1])
                        for h in range(HEADS):
                            ps = emit_scores(c, h, kts)
                            pending.append((c, h, ps, vs))
                            if len(pending) > 2:
                                emit_tail(*pending.pop(0))
                    for ent in pending:
                        emit_tail(*ent)

              # ---------------- P3: output projection (fp8 DR) ----------------
              if 3 in phases:
                with tc.tile_pool(name="otp", bufs=3) as otp, \
                   tc.tile_pool(name="so3", bufs=3) as so3, \
                   tc.tile_pool(name="pp3", bufs=4, space="PSUM") as pp3:
                    for tt in range(NTT):
                        ots = otp.tile([128, HEADS, 128], bf16, name="ots")
                        nc.sync.dma_start(ots[:], OTS[:, tt])
                        hilo = otp.tile([128, HEADS, 2, 128], f8, name="hilo")
                        nc.scalar.activation(hilo[:, :, 0], ots[:],
                                             AF.Copy, scale=SX)
                        nc.vector.scalar_tensor_tensor(
                            hilo[:, :, 1], ots[:], SX, hilo[:, :, 0],
                            ALU.mult, ALU.subtract)
                        so = so3.tile([128, DIMS], f32, name="so")
                        for nt in range(4):
                            ps = pp3.tile([128, 512], f32, name="pp3")
                            nsl = slice(nt * 512, (nt + 1) * 512)
                            for kg in range(8):
                                nc.tensor.matmul(
                                    ps[:], hilo[:, 2 * kg:2 * kg + 2, 0],
                                    wo[:, 2 * kg:2 * kg + 2, 1, nsl],
                                    start=(kg == 0), stop=False, perf_mode=DR)
                                nc.tensor.matmul(
                                    ps[:], hilo[:, 2 * kg, :],
                                    wo[:, 2 * kg, :, nsl],
                                    start=False, stop=False, perf_mode=DR)
                                nc.tensor.matmul(
                                    ps[:], hilo[:, 2 * kg + 1, :],
                                    wo[:, 2 * kg + 1, :, nsl],
                                    start=False, stop=(kg == 7), perf_mode=DR)
                            nc.scalar.activation(so[:, nsl], ps[:], AF.Copy,
                                                 scale=OSC)
                            if nt % 2 == 1:
                                nc.sync.dma_start(
                                    OUT[tt * 128:(tt + 1) * 128,
                                        (nt - 1) * 512:(nt + 1) * 512],
                                    so[:, (nt - 1) * 512:(nt + 1) * 512])
    return nc


def _split8(x, s):
    xs = np.asarray(x, np.float32) * np.float32(s)
    hi = xs.astype(E4M3)
    lo = (xs - hi.astype(np.float32)).astype(E4M3)
    return hi, lo


def _pack_w_qk(W):
    """[2048in, 2048out] -> [16 ob, 128 p, 16 g, 2 (lo,hi), 128 col] fp8."""
    hi, lo = _split8(W, SW)
    out = np.empty((16, 128, 16, 2, 128), E4M3)
    hi4 = hi.reshape(16, 128, 16, 128)   # [g, p, ob, col]
    lo4 = lo.reshape(16, 128, 16, 128)
    out[:, :, :, 0] = lo4.transpose(2, 1, 0, 3)
    out[:, :, :, 1] = hi4.transpose(2, 1, 0, 3)
    return np.ascontiguousarray(out)


def _pack_w_v(W):
    """[2048in, 2048out] -> [4 og, 128 p, 16 g, 2 (lo,hi), 512 col] fp8."""
    hi, lo = _split8(W, SW)
    out = np.empty((4, 128, 16, 2, 512), E4M3)
    hi4 = hi.reshape(16, 128, 4, 512)    # [g, p, og, col]
    lo4 = lo.reshape(16, 128, 4, 512)
    out[:, :, :, 0] = lo4.transpose(2, 1, 0, 3)
    out[:, :, :, 1] = hi4.transpose(2, 1, 0, 3)
    return np.ascontiguousarray(out)


def _pack_w_o(W):
    """[2048in, 2048out] -> [128 p, 16 g, 2 (lo,hi), 2048 col] fp8."""
    hi, lo = _split8(W, SW)
    out = np.empty((128, 16, 2, 2048), E4M3)
    out[:, :, 0] = lo.reshape(16, 128, 2048).transpose(1, 0, 2)
    out[:, :, 1] = hi.reshape(16, 128, 2048).transpose(1, 0, 2)
    return np.ascontiguousarray(out)


def _host_inputs(hidden_states, Wq, Wk, Wv, Wo, T):
    """Build the 8 per-core input maps."""
    TH = T + WIN
    inv_freq = 1.0 / (THETA ** (np.arange(0, HD, 2, dtype=np.float32) / HD))

    qq = np.arange(WIN)[None, :]
    kk = np.arange(128)[:, None]
    tri23 = np.concatenate([(qq >= kk), (qq >= kk + 128)], 1).astype(ml_dtypes.bfloat16)
    onesm_bf = np.ones((128, 128), ml_dtypes.bfloat16)

    wq_p = _pack_w_qk(Wq)
    wk_p = _pack_w_qk(Wk)
    wv_p = _pack_w_v(Wv)
    wo_p = _pack_w_o(Wo)

    in_maps = []
    for core in range(8):
        b, sh = divmod(core, NSH)
        t0 = sh * T
        hs = np.zeros((TH, DIMS), np.float32)
        lo_ = max(0, t0 - WIN)
        hs[WIN - (t0 - lo_):] = hidden_states[b, lo_:t0 + T]
        hT = hs.T                                         # [2048, TH]
        xh, xl = _split8(hT, SX)
        xhl = np.empty((128, 16, 2, TH), E4M3)
        xhl[:, :, 0] = xh.reshape(16, 128, TH).transpose(1, 0, 2)
        xhl[:, :, 1] = xl.reshape(16, 128, TH).transpose(1, 0, 2)

        pos = np.arange(t0 - WIN, t0 + T, dtype=np.float32)
        f = np.outer(inv_freq, pos)                      # [64, TH]
        cos = np.concatenate([np.cos(f), np.cos(f)], 0)  # [128, TH]
        sin = np.sin(f)
        sins = np.concatenate([-sin, sin], 0)
        pg = np.full((128, 1), -1e30 if sh == 0 else 0.0, np.float32)
        in_maps.append({
            "XHL": np.ascontiguousarray(xhl),
            "WQHL": wq_p, "WKHL": wk_p, "WVHL": wv_p, "WOHL": wo_p,
            "COSS": cos.astype(ml_dtypes.bfloat16),
            "SINS": sins.astype(ml_dtypes.bfloat16),
            "TRI23": tri23, "PGATE": pg, "ONESM": onesm_bf,
        })
    return in_maps


_CACHE = {}


def run(hidden_states, Wq, Wk, Wv, Wo, T=S // NSH, **spmd_kwargs):
    key = T
    if key not in _CACHE:
        nc = bacc.Bacc(None)
        build(nc, T)
        nc.finalize()
        _CACHE[key] = nc
    nc = _CACHE[key]
    in_maps = _host_inputs(hidden_states, Wq, Wk, Wv, Wo, T)
    res = run_bass_kernel_spmd(nc, in_maps, core_ids=list(range(8)), **spmd_kwargs)
    outs = [res.results[i]["OUT"] for i in range(8)]
    full = np.empty((B, NSH * T, DIMS), np.float32)
    for core in range(8):
        b, sh = divmod(core, NSH)
        full[b, sh * T:(sh + 1) * T] = outs[core]
    return full, res


def kernel(hidden_states, Wq, Wk, Wv, Wo):
    out, _ = run(np.asarray(hidden_states), Wq, Wk, Wv, Wo)
    return out


# revision 43
# speedup vs baseline: 1.0077x; 1.0077x over previous
"""Block sliding-window attention on 8 TRN2 NeuronCores.

Sharding: sequence-parallel, no collectives. 8 shards = (batch b in {0,1}) x
(quarter s in 0..3); each core owns 2048 consecutive tokens of one batch plus
a 256-token K/V halo from the previous quarter (zeros + -inf gate for the
first quarter).

All four projection matmuls run as fp8-e4m3 DoubleRow (2 contraction rows per
partition, 2x PE column rate) with a 3-term residual split for bf16-level
accuracy:  x@W ~= xh@Wh + xh@Wl + xl@Wh,  where xh = fp8(s*x) and
xl = fp8(s*x - xh) share one scale per tensor, so all three terms accumulate
in a single PSUM group (cost 12 col-cycles per 2048-deep column vs 16 bf16).

Per-core pipeline:
  P1: Q = x@Wq with RoPE fused on the PSUM path (ACT scaled copy -> bf16,
      partition-swap SBUF DMA for rotate-half, 3 DVE bf16 muls/adds with
      bf16 cos / pre-signed sin), roped Q kept resident in SBUF; roped K
      stored chunk-major to DRAM; V natural to DRAM.
  P2: per 256-token chunk and head: S^T = K Q^T per 128-key block into one
      [128,1024] PSUM tile, exp on ACT (scale 1/sqrt(128), -1e30 bias gates
      the no-previous case), 0/1 triangular mask multiply on DVE,
      denominator via all-ones matmul, O^T = V^T P^T, reciprocal on DVE,
      normalize on GpSimd(Pool), store O^T token-tile-major bf16.
  P3: out = O @ Wo via the same fp8 DoubleRow split (O quantized on the fly:
      ACT scaled copy for hi, fused DVE scalar_tensor_tensor for lo).
"""
import sys

try:
    import concourse  # noqa: F401
except ImportError:
    sys.path.insert(0, '/opt/trn_rl_repo')

import ml_dtypes
import numpy as np

import concourse.bacc as bacc
import concourse.mybir as mybir
import concourse.tile as tile
from concourse.bass_utils import run_bass_kernel_spmd

f32 = mybir.dt.float32
bf16 = mybir.dt.bfloat16
f8 = mybir.dt.float8e4
AF = mybir.ActivationFunctionType
ALU = mybir.AluOpType
DR = mybir.MatmulPerfMode.DoubleRow

DIMS = 2048
HEADS = 16
HD = 128           # head dim
WIN = 256          # window / chunk
B, S = 2, 8192
NSH = 4            # seq shards per batch
THETA = 10000.0
ISQ = float(1.0 / np.sqrt(HD))
SX = 16.0          # fp8 scale for activations (x, O)
SW = 1024.0        # fp8 scale for weights
OSC = float(1.0 / (SX * SW))
E4M3 = ml_dtypes.float8_e4m3


def tok_tiles(n):
    out, a = [], 0
    while a < n:
        w = min(512, n - a)
        out.append((a, w))
        a += w
    return out


def build(nc, T, phases=(1, 2, 3)):
    """Emit the per-core program. T = local tokens (multiple of 512)."""
    TH = T + WIN                      # with halo
    NC_ = T // WIN                    # chunks
    NTB = TH // 128                   # 128-token blocks incl halo
    NTT = T // 128                    # own 128-token blocks

    XHL = nc.dram_tensor("XHL", [128, 16, 2, TH], f8, kind="ExternalInput")
    WQHL = nc.dram_tensor("WQHL", [16, 128, 16, 2, 128], f8, kind="ExternalInput")
    WKHL = nc.dram_tensor("WKHL", [16, 128, 16, 2, 128], f8, kind="ExternalInput")
    WVHL = nc.dram_tensor("WVHL", [4, 128, 16, 2, 512], f8, kind="ExternalInput")
    WOHL = nc.dram_tensor("WOHL", [128, 16, 2, DIMS], f8, kind="ExternalInput")
    COSS = nc.dram_tensor("COSS", [128, TH], bf16, kind="ExternalInput")
    SINS = nc.dram_tensor("SINS", [128, TH], bf16, kind="ExternalInput")
    TRI23 = nc.dram_tensor("TRI23", [128, 2 * WIN], bf16, kind="ExternalInput")
    PGATE = nc.dram_tensor("PGATE", [128, 1], f32, kind="ExternalInput")
    ONESM = nc.dram_tensor("ONESM", [128, 128], bf16, kind="ExternalInput")
    OUT = nc.dram_tensor("OUT", [T, DIMS], f32, kind="ExternalOutput")

    KTS = nc.dram_tensor("KTS", [128, NC_ + 1, HEADS, WIN], bf16)  # chunk-major roped K^T
    VS = nc.dram_tensor("VS", [TH, DIMS], bf16)                    # V natural
    OTS = nc.dram_tensor("OTS", [128, NTT, HEADS, 128], bf16)      # tt-major normalized O^T

    with tile.TileContext(nc) as tc:
        with tc.tile_pool(name="cst", bufs=1) as cst, \
             tc.tile_pool(name="qres", bufs=1) as qres:
            tri23 = cst.tile([128, 2 * WIN], bf16)
            pgate = cst.tile([128, 1], f32)
            onesm = cst.tile([128, 128], bf16)
            nc.scalar.dma_start(tri23[:], TRI23[:])
            nc.scalar.dma_start(pgate[:], PGATE[:])
            nc.scalar.dma_start(onesm[:], ONESM[:])
            qts = qres.tile([128, HEADS, T], bf16)   # roped Q^T, SBUF-resident

            # ---------------- P1: projections + RoPE ----------------
            if 1 in phases:
              with tc.tile_pool(name="p1x", bufs=1) as p1x, \
                 tc.tile_pool(name="trig", bufs=1) as trig, \
                 tc.tile_pool(name="wp", bufs=2) as wp, \
                 tc.tile_pool(name="wvp", bufs=2) as wvp, \
                 tc.tile_pool(name="vbp", bufs=3) as vbp, \
                 tc.tile_pool(name="rp", bufs=3) as rp, \
                 tc.tile_pool(name="pp", bufs=5, space="PSUM") as pp, \
                 tc.tile_pool(name="ppv", bufs=3, space="PSUM") as ppv:
                xhl = p1x.tile([128, 16, 2, TH], f8)
                wt0 = wp.tile([128, 16, 2, 128], f8, name="wt")
                with tc.high_priority():
                    nc.sync.dma_start(wt0[:], WQHL[0])
                for hl in (0, 1):
                    for g in range(0, 16, 2):
                        nc.sync.dma_start(xhl[:, g:g + 2, hl],
                                          XHL[:, g:g + 2, hl])
                coss = trig.tile([128, TH], bf16)
                sins = trig.tile([128, TH], bf16)
                nc.scalar.dma_start(coss[:], COSS[:])
                nc.scalar.dma_start(sins[:], SINS[:])

                # Q (toff=WIN, RoPE pos=WIN+a, dest SBUF qts) and
                # K (toff=0, RoPE pos=a, dest DRAM KTS chunk-major)
                for W_, toff, tlen, is_q in ((WQHL, WIN, T, True),
                                             (WKHL, 0, TH, False)):
                    for ob in range(HEADS):
                        if is_q and ob == 0:
                            wt = wt0
                        else:
                            wt = wp.tile([128, 16, 2, 128], f8, name="wt")
                            nc.sync.dma_start(wt[:], W_[ob])
                        for a, w in tok_tiles(tlen):
                            ps = pp.tile([128, 512], f32, name="ps")
                            tsl = slice(toff + a, toff + a + w)
                            for kg in range(8):
                                nc.tensor.matmul(
                                    ps[:, :w], wt[:, 2 * kg:2 * kg + 2, 1],
                                    xhl[:, 2 * kg:2 * kg + 2, 0, tsl],
                                    start=(kg == 0), stop=False, perf_mode=DR)
                            for g in range(16):
                                nc.tensor.matmul(
                                    ps[:, :w], wt[:, g, :],
                                    xhl[:, g, :, tsl],
                                    start=False, stop=(g == 15), perf_mode=DR)
                            raw = rp.tile([128, 512], bf16, name="raw")
                            nc.scalar.activation(raw[:, :w], ps[:, :w],
                                                 AF.Copy, scale=OSC)
                            rot = rp.tile([128, 512], bf16, name="rot")
                            nc.sync.dma_start(rot[0:64, :w], raw[64:128, :w])
                            nc.sync.dma_start(rot[64:128, :w], raw[0:64, :w])
                            pos = toff + a
                            t1 = rp.tile([128, 512], bf16, name="t1")
                            nc.vector.tensor_mul(t1[:, :w], rot[:, :w],
                                                 sins[:, pos:pos + w])
                            t2 = rp.tile([128, 512], bf16, name="t2")
                            nc.vector.tensor_mul(t2[:, :w], raw[:, :w],
                                                 coss[:, pos:pos + w])
                            if is_q:
                                nc.vector.tensor_add(
                                    qts[:, ob, a:a + w], t2[:, :w], t1[:, :w])
                            else:
                                rop = rp.tile([128, 512], bf16, name="rop")
                                nc.vector.tensor_add(rop[:, :w], t2[:, :w],
                                                     t1[:, :w])
                                c0 = a // WIN
                                nw = w // WIN
                                nc.sync.dma_start(
                                    KTS[:, c0:c0 + nw, ob], rop[:, :w])

                # V natural: lhsT = x block [128in(x2), 128tok], rhs = Wv rows
                for og in range(4):
                    wv = wvp.tile([128, 16, 2, 512], f8, name="wv")
                    nc.sync.dma_start(wv[:], WVHL[og])
                    for tb in range(NTB):
                        ps = ppv.tile([128, 512], f32, name="psv")
                        tsl = slice(tb * 128, (tb + 1) * 128)
                        for kg in range(8):
                            nc.tensor.matmul(
                                ps[:], xhl[:, 2 * kg:2 * kg + 2, 0, tsl],
                                wv[:, 2 * kg:2 * kg + 2, 1],
                                start=(kg == 0), stop=False, perf_mode=DR)
                            nc.tensor.matmul(
                                ps[:], xhl[:, 2 * kg, :, tsl],
                                wv[:, 2 * kg, :],
                                start=False, stop=False, perf_mode=DR)
                            nc.tensor.matmul(
                                ps[:], xhl[:, 2 * kg + 1, :, tsl],
                                wv[:, 2 * kg + 1, :],
                                start=False, stop=(kg == 7), perf_mode=DR)
                        vb = vbp.tile([128, 512], bf16, name="vb")
                        nc.vector.tensor_scalar_mul(vb[:], ps[:], OSC)
                        nc.sync.dma_start(
                            VS[tb * 128:(tb + 1) * 128,
                               og * 512:(og + 1) * 512], vb[:])

            # ---------------- P3 weight prefetch + P2: attention ----------------
            with tc.tile_pool(name="wo3", bufs=1) as wo3:
              wo = wo3.tile([128, 16, 2, DIMS], f8)

              if 2 in phases:
                with tc.tile_pool(name="ktp", bufs=3) as ktp, \
                   tc.tile_pool(name="qk", bufs=4) as qk, \
                   tc.tile_pool(name="pbp", bufs=3) as pbp, \
                   tc.tile_pool(name="obp", bufs=4) as obp, \
                   tc.tile_pool(name="ps_s", bufs=3, space="PSUM") as ps_s, \
                   tc.tile_pool(name="ps_dpo", bufs=2, space="PSUM") as ps_dpo:
                    def kload(c):
                        kt = ktp.tile([128, HEADS, WIN], bf16, name="kt")
                        nc.sync.dma_start(kt[:], KTS[:, c])
                        return kt

                    def vload(c):
                        v = qk.tile([128, 2, DIMS], bf16, name="v")
                        nc.sync.dma_start(
                            v[:], VS[c * WIN:(c + 1) * WIN]
                            .rearrange("(tb p) c -> p tb c", p=128))
                        return v

                    kt_t = [kload(0), kload(1)]
                    v_t = [vload(0), vload(1)]

                    # S^T layout [keys, queries], 896 cols per head:
                    # kb0 [0:256], kb1 [256:512], kb2 [512:768] (left 128
                    # triangular), kb3-right [768:896] (triangular; the
                    # fully-masked kb3 x q<128 quadrant is never computed).
                    NSC = 3 * WIN + 128
                    def emit_scores(c, h, kts):
                        ps = ps_s.tile([128, NSC], f32, name="ps")
                        for kb in range(3):
                            nc.tensor.matmul(
                                ps[:, kb * WIN:(kb + 1) * WIN],
                                kts[kb // 2][:, h, (kb % 2) * 128:(kb % 2) * 128 + 128],
                                qts[:, h, c * WIN:(c + 1) * WIN],
                                start=True, stop=True)
                        nc.tensor.matmul(
                            ps[:, 3 * WIN:], kts[1][:, h, 128:256],
                            qts[:, h, c * WIN + 128:(c + 1) * WIN],
                            start=True, stop=True)
                        return ps

                    def emit_tail(c, h, ps, vs):
                        pb = pbp.tile([128, NSC], bf16, name="pb")
                        if c == 0:
                            nc.scalar.activation(pb[:, 0:2 * WIN], ps[:, 0:2 * WIN],
                                                 AF.Exp, bias=pgate[:], scale=ISQ)
                            nc.scalar.activation(pb[:, 2 * WIN:], ps[:, 2 * WIN:],
                                                 AF.Exp, scale=ISQ)
                        else:
                            nc.scalar.activation(pb[:], ps[:], AF.Exp, scale=ISQ)
                        tri1 = tri23[:, 0:128]
                        nc.gpsimd.tensor_mul(pb[:, 512:640], pb[:, 512:640], tri1)
                        nc.gpsimd.tensor_mul(pb[:, 768:896], pb[:, 768:896], tri1)
                        pdo = ps_dpo.tile([128, 2 * WIN], f32, name="pdo")
                        pd = pdo[:, 0:WIN]
                        po = pdo[:, WIN:2 * WIN]
                        for kb in range(3):
                            nc.tensor.matmul(
                                pd, onesm[:], pb[:, kb * WIN:(kb + 1) * WIN],
                                start=(kb == 0), stop=False)
                        nc.tensor.matmul(
                            pdo[:, 128:WIN], onesm[:], pb[:, 3 * WIN:],
                            start=False, stop=True, skip_group_check=True)
                        for kb in range(3):
                            nc.tensor.matmul(
                                po, vs[kb // 2][:, kb % 2, h * 128:(h + 1) * 128],
                                pb[:, kb * WIN:(kb + 1) * WIN],
                                start=(kb == 0), stop=False)
                        nc.tensor.matmul(
                            pdo[:, WIN + 128:2 * WIN],
                            vs[1][:, 1, h * 128:(h + 1) * 128],
                            pb[:, 3 * WIN:],
                            start=False, stop=True, skip_group_check=True)
                        rb = obp.tile([128, WIN], f32, name="rb")
                        with nc.allow_low_precision("softmax denominator"):
                            nc.vector.reciprocal(rb[:], pd)
                        ot = obp.tile([128, WIN], bf16, name="ot")
                        nc.vector.tensor_mul(ot[:], po, rb[:])
                        nc.sync.dma_start(OTS[:, 2 * c:2 * c + 2, h], ot[:])

                    pending = []
                    for c in range(NC_):
                        if c + 2 <= NC_:
                            kt_t.append(kload(c + 2))
                            v_t.append(vload(c + 2))
                        if c >= 1:
                            per = -(-16 // max(1, NC_ - 1))
                            for g in range(per * (c - 1),
                                           min(16, per * c)):
                                nc.sync.dma_start(wo[:, g], WOHL[:, g])
                        kts = (kt_t[c], kt_t[c + 1])
                        vs = (v_t[c], v_t[c + 1])
                        for h in range(HEADS):
                            ps = emit_scores(c, h, kts)
                            pending.append((c, h, ps, vs))
                            if len(pending) > 2:
                                emit_tail(*pending.pop(0))
                    for ent in pending:
                        emit_tail(*ent)

              # ---------------- P3: output projection (fp8 DR) ----------------
              if 3 in phases:
                with tc.tile_pool(name="otp", bufs=3) as otp, \
                   tc.tile_pool(name="so3", bufs=3) as so3, \
                   tc.tile_pool(name="pp3", bufs=4, space="PSUM") as pp3:
                    for tt in range(NTT):
                        ots = otp.tile([128, HEADS, 128], bf16, name="ots")
                        nc.sync.dma_start(ots[:], OTS[:, tt])
                        hilo = otp.tile([128, HEADS, 2, 128], f8, name="hilo")
                        nc.scalar.activation(hilo[:, :, 0], ots[:],
                                             AF.Copy, scale=SX)
                        nc.vector.scalar_tensor_tensor(
                            hilo[:, :, 1], ots[:], SX, hilo[:, :, 0],
                            ALU.mult, ALU.subtract)
                        so = so3.tile([128, DIMS], f32, name="so")
                        for nt in range(4):
                            ps = pp3.tile([128, 512], f32, name="pp3")
                            nsl = slice(nt * 512, (nt + 1) * 512)
                            for kg in range(8):
                                nc.tensor.matmul(
                                    ps[:], hilo[:, 2 * kg:2 * kg + 2, 0],
                                    wo[:, 2 * kg:2 * kg + 2, 1, nsl],
                                    start=(kg == 0), stop=False, perf_mode=DR)
                                nc.tensor.matmul(
                                    ps[:], hilo[:, 2 * kg, :],
                                    wo[:, 2 * kg, :, nsl],
                                    start=False, stop=False, perf_mode=DR)
                                nc.tensor.matmul(
                                    ps[:], hilo[:, 2 * kg + 1, :],
                                    wo[:, 2 * kg + 1, :, nsl],
                                    start=False, stop=(kg == 7), perf_mode=DR)
                            nc.scalar.activation(so[:, nsl], ps[:], AF.Copy,
                                                 scale=OSC)
                            if nt % 2 == 1:
                                nc.sync.dma_start(
                                    OUT[tt * 128:(tt + 1) * 128,
                                        (nt - 1) * 512:(nt + 1) * 512],
                                    so[:, (nt - 1) * 512:(nt + 1) * 512])
    return nc


def _split8(x, s):
    xs = np.asarray(x, np.float32) * np.float32(s)
    hi = xs.astype(E4M3)
    lo = (xs - hi.astype(np.float32)).astype(E4M3)
    return hi, lo


def _pack_w_qk(W):
    """[2048in, 2048out] -> [16 ob, 128 p, 16 g, 2 (lo,hi), 128 col] fp8."""
    hi, lo = _split8(W, SW)
    out = np.empty((16, 128, 16, 2, 128), E4M3)
    hi4 = hi.reshape(16, 128, 16, 128)   # [g, p, ob, col]
    lo4 = lo.reshape(16, 128, 16, 128)
    out[:, :, :, 0] = lo4.transpose(2, 1, 0, 3)
    out[:, :, :, 1] = hi4.transpose(2, 1, 0, 3)
    return np.ascontiguousarray(out)


def _pack_w_v(W):
    """[2048in, 2048out] -> [4 og, 128 p, 16 g, 2 (lo,hi), 512 col] fp8."""
    hi, lo = _split8(W, SW)
    out = np.empty((4, 128, 16, 2, 512), E4M3)
    hi4 = hi.reshape(16, 128, 4, 512)    # [g, p, og, col]
    lo4 = lo.reshape(16, 128, 4, 512)
    out[:, :, :, 0] = lo4.transpose(2, 1, 0, 3)
    out[:, :, :, 1] = hi4.transpose(2, 1, 0, 3)
    return np.ascontiguousarray(out)


def _pack_w_o(W):
    """[2048in, 2048out] -> [128 p, 16 g, 2 (lo,hi), 2048 col] fp8."""
    hi, lo = _split8(W, SW)
    out = np.empty((128, 16, 2, 2048), E4M3)
    out[:, :, 0] = lo.reshape(16, 128, 2048).transpose(1, 0, 2)
    out[:, :, 1] = hi.reshape(16, 128, 2048).transpose(1, 0, 2)
    return np.ascontiguousarray(out)


def _host_inputs(hidden_states, Wq, Wk, Wv, Wo, T):
    """Build the 8 per-core input maps."""
    TH = T + WIN
    inv_freq = 1.0 / (THETA ** (np.arange(0, HD, 2, dtype=np.float32) / HD))

    qq = np.arange(WIN)[None, :]
    kk = np.arange(128)[:, None]
    tri23 = np.concatenate([(qq >= kk), (qq >= kk + 128)], 1).astype(ml_dtypes.bfloat16)
    onesm_bf = np.ones((128, 128), ml_dtypes.bfloat16)

    wq_p = _pack_w_qk(Wq)
    wk_p = _pack_w_qk(Wk)
    wv_p = _pack_w_v(Wv)
    wo_p = _pack_w_o(Wo)

    in_maps = []
    for core in range(8):
        b, sh = divmod(core, NSH)
        t0 = sh * T
        hs = np.zeros((TH, DIMS), np.float32)
        lo_ = max(0, t0 - WIN)
        hs[WIN - (t0 - lo_):] = hidden_states[b, lo_:t0 + T]
        hT = hs.T                                         # [2048, TH]
        xh, xl = _split8(hT, SX)
        xhl = np.empty((128, 16, 2, TH), E4M3)
        xhl[:, :, 0] = xh.reshape(16, 128, TH).transpose(1, 0, 2)
        xhl[:, :, 1] = xl.reshape(16, 128, TH).transpose(1, 0, 2)

        pos = np.arange(t0 - WIN, t0 + T, dtype=np.float32)
        f = np.outer(inv_freq, pos)                      # [64, TH]
        cos = np.concatenate([np.cos(f), np.cos(f)], 0)  # [128, TH]
        sin = np.sin(f)
        sins = np.concatenate([-sin, sin], 0)
        pg = np.full((128, 1), -1e30 if sh == 0 else 0.0, np.float32)
        in_maps.append({
            "XHL": np.ascontiguousarray(xhl),
            "WQHL": wq_p, "WKHL": wk_p, "WVHL": wv_p, "WOHL": wo_p,
            "COSS": cos.astype(ml_dtypes.bfloat16),
            "SINS": sins.astype(ml_dtypes.bfloat16),
            "TRI23": tri23, "PGATE": pg, "ONESM": onesm_bf,
        })
    return in_maps


_CACHE = {}


def run(hidden_states, Wq, Wk, Wv, Wo, T=S // NSH, **spmd_kwargs):
    key = T
    if key not in _CACHE:
        nc = bacc.Bacc(None)
        build(nc, T)
        nc.finalize()
        _CACHE[key] = nc
    nc = _CACHE[key]
    in_maps = _host_inputs(hidden_states, Wq, Wk, Wv, Wo, T)
    res = run_bass_kernel_spmd(nc, in_maps, core_ids=list(range(8)), **spmd_kwargs)
    outs = [res.results[i]["OUT"] for i in range(8)]
    full = np.empty((B, NSH * T, DIMS), np.float32)
    for core in range(8):
        b, sh = divmod(core, NSH)
        full[b, sh * T:(sh + 1) * T] = outs[core]
    return full, res


def kernel(hidden_states, Wq, Wk, Wv, Wo):
    out, _ = run(np.asarray(hidden_states), Wq, Wk, Wv, Wo)
    return out


# revision 44
# speedup vs baseline: 1.0106x; 1.0029x over previous
"""Block sliding-window attention on 8 TRN2 NeuronCores.

Sharding: sequence-parallel, no collectives. 8 shards = (batch b in {0,1}) x
(quarter s in 0..3); each core owns 2048 consecutive tokens of one batch plus
a 256-token K/V halo from the previous quarter (zeros + -inf gate for the
first quarter).

All four projection matmuls run as fp8-e4m3 DoubleRow (2 contraction rows per
partition, 2x PE column rate) with a 3-term residual split for bf16-level
accuracy:  x@W ~= xh@Wh + xh@Wl + xl@Wh,  where xh = fp8(s*x) and
xl = fp8(s*x - xh) share one scale per tensor, so all three terms accumulate
in a single PSUM group (cost 12 col-cycles per 2048-deep column vs 16 bf16).

Per-core pipeline:
  P1: Q = x@Wq with RoPE fused on the PSUM path (ACT scaled copy -> bf16,
      partition-swap SBUF DMA for rotate-half, 3 DVE bf16 muls/adds with
      bf16 cos / pre-signed sin), roped Q kept resident in SBUF; roped K
      stored chunk-major to DRAM; V natural to DRAM.
  P2: per 256-token chunk and head: S^T = K Q^T per 128-key block into one
      [128,1024] PSUM tile, exp on ACT (scale 1/sqrt(128), -1e30 bias gates
      the no-previous case), 0/1 triangular mask multiply on DVE,
      denominator via all-ones matmul, O^T = V^T P^T, reciprocal on DVE,
      normalize on GpSimd(Pool), store O^T token-tile-major bf16.
  P3: out = O @ Wo via the same fp8 DoubleRow split (O quantized on the fly:
      ACT scaled copy for hi, fused DVE scalar_tensor_tensor for lo).
"""
import sys

try:
    import concourse  # noqa: F401
except ImportError:
    sys.path.insert(0, '/opt/trn_rl_repo')

import ml_dtypes
import numpy as np

import concourse.bacc as bacc
import concourse.mybir as mybir
import concourse.tile as tile
from concourse.bass_utils import run_bass_kernel_spmd

f32 = mybir.dt.float32
bf16 = mybir.dt.bfloat16
f8 = mybir.dt.float8e4
AF = mybir.ActivationFunctionType
ALU = mybir.AluOpType
DR = mybir.MatmulPerfMode.DoubleRow

DIMS = 2048
HEADS = 16
HD = 128           # head dim
WIN = 256          # window / chunk
B, S = 2, 8192
NSH = 4            # seq shards per batch
THETA = 10000.0
ISQ = float(1.0 / np.sqrt(HD))
SX = 16.0          # fp8 scale for activations (x, O)
SW = 1024.0        # fp8 scale for weights
OSC = float(1.0 / (SX * SW))
E4M3 = ml_dtypes.float8_e4m3


def tok_tiles(n):
    out, a = [], 0
    while a < n:
        w = min(512, n - a)
        out.append((a, w))
        a += w
    return out


def build(nc, T, phases=(1, 2, 3)):
    """Emit the per-core program. T = local tokens (multiple of 512)."""
    TH = T + WIN                      # with halo
    NC_ = T // WIN                    # chunks
    NTB = TH // 128                   # 128-token blocks incl halo
    NTT = T // 128                    # own 128-token blocks

    XHL = nc.dram_tensor("XHL", [128, 16, 2, TH], f8, kind="ExternalInput")
    WQHL = nc.dram_tensor("WQHL", [16, 128, 16, 2, 128], f8, kind="ExternalInput")
    WKHL = nc.dram_tensor("WKHL", [16, 128, 16, 2, 128], f8, kind="ExternalInput")
    WVHL = nc.dram_tensor("WVHL", [4, 128, 16, 2, 512], f8, kind="ExternalInput")
    WOHL = nc.dram_tensor("WOHL", [128, 16, 2, DIMS], f8, kind="ExternalInput")
    COSS = nc.dram_tensor("COSS", [128, TH], bf16, kind="ExternalInput")
    SINS = nc.dram_tensor("SINS", [128, TH], bf16, kind="ExternalInput")
    TRI23 = nc.dram_tensor("TRI23", [128, 2 * WIN], bf16, kind="ExternalInput")
    PGATE = nc.dram_tensor("PGATE", [128, 1], f32, kind="ExternalInput")
    ONESM = nc.dram_tensor("ONESM", [128, 128], bf16, kind="ExternalInput")
    OUT = nc.dram_tensor("OUT", [T, DIMS], f32, kind="ExternalOutput")

    KTS = nc.dram_tensor("KTS", [128, NC_ + 1, HEADS, WIN], bf16)  # chunk-major roped K^T
    VS = nc.dram_tensor("VS", [TH, DIMS], bf16)                    # V natural
    OTS = nc.dram_tensor("OTS", [128, NTT, HEADS, 128], bf16)      # tt-major normalized O^T

    with tile.TileContext(nc) as tc:
        with tc.tile_pool(name="cst", bufs=1) as cst, \
             tc.tile_pool(name="qres", bufs=1) as qres:
            tri23 = cst.tile([128, 2 * WIN], bf16)
            pgate = cst.tile([128, 1], f32)
            onesm = cst.tile([128, 128], bf16)
            nc.scalar.dma_start(tri23[:], TRI23[:])
            nc.scalar.dma_start(pgate[:], PGATE[:])
            nc.scalar.dma_start(onesm[:], ONESM[:])
            qts = qres.tile([128, HEADS, T], bf16)   # roped Q^T, SBUF-resident

            # ---------------- P1: projections + RoPE ----------------
            if 1 in phases:
              with tc.tile_pool(name="p1x", bufs=1) as p1x, \
                 tc.tile_pool(name="trig", bufs=1) as trig, \
                 tc.tile_pool(name="wp", bufs=2) as wp, \
                 tc.tile_pool(name="wvp", bufs=2) as wvp, \
                 tc.tile_pool(name="vbp", bufs=3) as vbp, \
                 tc.tile_pool(name="rp", bufs=3) as rp, \
                 tc.tile_pool(name="pp", bufs=4, space="PSUM") as pp, \
                 tc.tile_pool(name="ppv", bufs=4, space="PSUM") as ppv:
                xhl = p1x.tile([128, 16, 2, TH], f8)
                wt0 = wp.tile([128, 16, 2, 128], f8, name="wt")
                with tc.high_priority():
                    nc.sync.dma_start(wt0[:], WQHL[0])
                for hl in (0, 1):
                    for g in range(0, 16, 2):
                        nc.sync.dma_start(xhl[:, g:g + 2, hl],
                                          XHL[:, g:g + 2, hl])
                coss = trig.tile([128, TH], bf16)
                sins = trig.tile([128, TH], bf16)
                nc.scalar.dma_start(coss[:], COSS[:])
                nc.scalar.dma_start(sins[:], SINS[:])

                # Q (toff=WIN, RoPE pos=WIN+a, dest SBUF qts) and
                # K (toff=0, RoPE pos=a, dest DRAM KTS chunk-major)
                for W_, toff, tlen, is_q in ((WQHL, WIN, T, True),
                                             (WKHL, 0, TH, False)):
                    for ob in range(HEADS):
                        if is_q and ob == 0:
                            wt = wt0
                        else:
                            wt = wp.tile([128, 16, 2, 128], f8, name="wt")
                            nc.sync.dma_start(wt[:], W_[ob])
                        for a, w in tok_tiles(tlen):
                            ps = pp.tile([128, 512], f32, name="ps")
                            tsl = slice(toff + a, toff + a + w)
                            for kg in range(8):
                                nc.tensor.matmul(
                                    ps[:, :w], wt[:, 2 * kg:2 * kg + 2, 1],
                                    xhl[:, 2 * kg:2 * kg + 2, 0, tsl],
                                    start=(kg == 0), stop=False, perf_mode=DR)
                            for g in range(16):
                                nc.tensor.matmul(
                                    ps[:, :w], wt[:, g, :],
                                    xhl[:, g, :, tsl],
                                    start=False, stop=(g == 15), perf_mode=DR)
                            raw = rp.tile([128, 512], bf16, name="raw")
                            nc.scalar.activation(raw[:, :w], ps[:, :w],
                                                 AF.Copy, scale=OSC)
                            rot = rp.tile([128, 512], bf16, name="rot")
                            nc.sync.dma_start(rot[0:64, :w], raw[64:128, :w])
                            nc.sync.dma_start(rot[64:128, :w], raw[0:64, :w])
                            pos = toff + a
                            t1 = rp.tile([128, 512], bf16, name="t1")
                            nc.vector.tensor_mul(t1[:, :w], rot[:, :w],
                                                 sins[:, pos:pos + w])
                            t2 = rp.tile([128, 512], bf16, name="t2")
                            nc.vector.tensor_mul(t2[:, :w], raw[:, :w],
                                                 coss[:, pos:pos + w])
                            if is_q:
                                nc.vector.tensor_add(
                                    qts[:, ob, a:a + w], t2[:, :w], t1[:, :w])
                            else:
                                rop = rp.tile([128, 512], bf16, name="rop")
                                nc.vector.tensor_add(rop[:, :w], t2[:, :w],
                                                     t1[:, :w])
                                c0 = a // WIN
                                nw = w // WIN
                                nc.sync.dma_start(
                                    KTS[:, c0:c0 + nw, ob], rop[:, :w])

                # V natural: lhsT = x block [128in(x2), 128tok], rhs = Wv rows
                for og in range(4):
                    wv = wvp.tile([128, 16, 2, 512], f8, name="wv")
                    nc.sync.dma_start(wv[:], WVHL[og])
                    for tb in range(NTB):
                        ps = ppv.tile([128, 512], f32, name="psv")
                        tsl = slice(tb * 128, (tb + 1) * 128)
                        for kg in range(8):
                            nc.tensor.matmul(
                                ps[:], xhl[:, 2 * kg:2 * kg + 2, 0, tsl],
                                wv[:, 2 * kg:2 * kg + 2, 1],
                                start=(kg == 0), stop=False, perf_mode=DR)
                            nc.tensor.matmul(
                                ps[:], xhl[:, 2 * kg, :, tsl],
                                wv[:, 2 * kg, :],
                                start=False, stop=False, perf_mode=DR)
                            nc.tensor.matmul(
                                ps[:], xhl[:, 2 * kg + 1, :, tsl],
                                wv[:, 2 * kg + 1, :],
                                start=False, stop=(kg == 7), perf_mode=DR)
                        vb = vbp.tile([128, 512], bf16, name="vb")
                        nc.vector.tensor_scalar_mul(vb[:], ps[:], OSC)
                        nc.sync.dma_start(
                            VS[tb * 128:(tb + 1) * 128,
                               og * 512:(og + 1) * 512], vb[:])

            # ---------------- P3 weight prefetch + P2: attention ----------------
            with tc.tile_pool(name="wo3", bufs=1) as wo3:
              wo = wo3.tile([128, 16, 2, DIMS], f8)

              if 2 in phases:
                with tc.tile_pool(name="ktp", bufs=3) as ktp, \
                   tc.tile_pool(name="qk", bufs=4) as qk, \
                   tc.tile_pool(name="pbp", bufs=3) as pbp, \
                   tc.tile_pool(name="obp", bufs=4) as obp, \
                   tc.tile_pool(name="ps_s", bufs=3, space="PSUM") as ps_s, \
                   tc.tile_pool(name="ps_dpo", bufs=2, space="PSUM") as ps_dpo:
                    def kload(c):
                        kt = ktp.tile([128, HEADS, WIN], bf16, name="kt")
                        nc.sync.dma_start(kt[:], KTS[:, c])
                        return kt

                    def vload(c):
                        v = qk.tile([128, 2, DIMS], bf16, name="v")
                        nc.sync.dma_start(
                            v[:], VS[c * WIN:(c + 1) * WIN]
                            .rearrange("(tb p) c -> p tb c", p=128))
                        return v

                    kt_t = [kload(0), kload(1)]
                    v_t = [vload(0), vload(1)]

                    # S^T layout [keys, queries], 896 cols per head:
                    # kb0 [0:256], kb1 [256:512], kb2 [512:768] (left 128
                    # triangular), kb3-right [768:896] (triangular; the
                    # fully-masked kb3 x q<128 quadrant is never computed).
                    NSC = 3 * WIN + 128
                    def emit_scores(c, h, kts):
                        ps = ps_s.tile([128, NSC], f32, name="ps")
                        for kb in range(3):
                            nc.tensor.matmul(
                                ps[:, kb * WIN:(kb + 1) * WIN],
                                kts[kb // 2][:, h, (kb % 2) * 128:(kb % 2) * 128 + 128],
                                qts[:, h, c * WIN:(c + 1) * WIN],
                                start=True, stop=True)
                        nc.tensor.matmul(
                            ps[:, 3 * WIN:], kts[1][:, h, 128:256],
                            qts[:, h, c * WIN + 128:(c + 1) * WIN],
                            start=True, stop=True)
                        return ps

                    def emit_tail(c, h, ps, vs):
                        pb = pbp.tile([128, NSC], bf16, name="pb")
                        if c == 0:
                            nc.scalar.activation(pb[:, 0:2 * WIN], ps[:, 0:2 * WIN],
                                                 AF.Exp, bias=pgate[:], scale=ISQ)
                            nc.scalar.activation(pb[:, 2 * WIN:], ps[:, 2 * WIN:],
                                                 AF.Exp, scale=ISQ)
                        else:
                            nc.scalar.activation(pb[:], ps[:], AF.Exp, scale=ISQ)
                        tri1 = tri23[:, 0:128]
                        nc.gpsimd.tensor_mul(pb[:, 512:640], pb[:, 512:640], tri1)
                        nc.gpsimd.tensor_mul(pb[:, 768:896], pb[:, 768:896], tri1)
                        pdo = ps_dpo.tile([128, 2 * WIN], f32, name="pdo")
                        pd = pdo[:, 0:WIN]
                        po = pdo[:, WIN:2 * WIN]
                        for kb in range(3):
                            nc.tensor.matmul(
                                pd, onesm[:], pb[:, kb * WIN:(kb + 1) * WIN],
                                start=(kb == 0), stop=False)
                        nc.tensor.matmul(
                            pdo[:, 128:WIN], onesm[:], pb[:, 3 * WIN:],
                            start=False, stop=True, skip_group_check=True)
                        for kb in range(3):
                            nc.tensor.matmul(
                                po, vs[kb // 2][:, kb % 2, h * 128:(h + 1) * 128],
                                pb[:, kb * WIN:(kb + 1) * WIN],
                                start=(kb == 0), stop=False)
                        nc.tensor.matmul(
                            pdo[:, WIN + 128:2 * WIN],
                            vs[1][:, 1, h * 128:(h + 1) * 128],
                            pb[:, 3 * WIN:],
                            start=False, stop=True, skip_group_check=True)
                        rb = obp.tile([128, WIN], f32, name="rb")
                        with nc.allow_low_precision("softmax denominator"):
                            nc.vector.reciprocal(rb[:], pd)
                        ot = obp.tile([128, WIN], bf16, name="ot")
                        nc.vector.tensor_mul(ot[:], po, rb[:])
                        nc.sync.dma_start(OTS[:, 2 * c:2 * c + 2, h], ot[:])

                    pending = []
                    for c in range(NC_):
                        if c + 2 <= NC_:
                            kt_t.append(kload(c + 2))
                            v_t.append(vload(c + 2))
                        if c >= 1:
                            per = -(-16 // max(1, NC_ - 1))
                            for g in range(per * (c - 1),
                                           min(16, per * c)):
                                nc.sync.dma_start(wo[:, g], WOHL[:, g])
                        kts = (kt_t[c], kt_t[c + 1])
                        vs = (v_t[c], v_t[c + 1])
                        for h in range(HEADS):
                            ps = emit_scores(c, h, kts)
                            pending.append((c, h, ps, vs))
                            if len(pending) > 2:
                                emit_tail(*pending.pop(0))
                    for ent in pending:
                        emit_tail(*ent)

              # ---------------- P3: output projection (fp8 DR) ----------------
              if 3 in phases:
                with tc.tile_pool(name="otp", bufs=3) as otp, \
                   tc.tile_pool(name="so3", bufs=3) as so3, \
                   tc.tile_pool(name="pp3", bufs=4, space="PSUM") as pp3:
                    for tt in range(NTT):
                        ots = otp.tile([128, HEADS, 128], bf16, name="ots")
                        nc.sync.dma_start(ots[:], OTS[:, tt])
                        hilo = otp.tile([128, HEADS, 2, 128], f8, name="hilo")
                        nc.scalar.activation(hilo[:, :, 0], ots[:],
                                             AF.Copy, scale=SX)
                        nc.vector.scalar_tensor_tensor(
                            hilo[:, :, 1], ots[:], SX, hilo[:, :, 0],
                            ALU.mult, ALU.subtract)
                        so = so3.tile([128, DIMS], f32, name="so")
                        for nt in range(4):
                            ps = pp3.tile([128, 512], f32, name="pp3")
                            nsl = slice(nt * 512, (nt + 1) * 512)
                            for kg in range(8):
                                nc.tensor.matmul(
                                    ps[:], hilo[:, 2 * kg:2 * kg + 2, 0],
                                    wo[:, 2 * kg:2 * kg + 2, 1, nsl],
                                    start=(kg == 0), stop=False, perf_mode=DR)
                                nc.tensor.matmul(
                                    ps[:], hilo[:, 2 * kg, :],
                                    wo[:, 2 * kg, :, nsl],
                                    start=False, stop=False, perf_mode=DR)
                                nc.tensor.matmul(
                                    ps[:], hilo[:, 2 * kg + 1, :],
                                    wo[:, 2 * kg + 1, :, nsl],
                                    start=False, stop=(kg == 7), perf_mode=DR)
                            nc.scalar.activation(so[:, nsl], ps[:], AF.Copy,
                                                 scale=OSC)
                            if nt % 2 == 1:
                                nc.sync.dma_start(
                                    OUT[tt * 128:(tt + 1) * 128,
                                        (nt - 1) * 512:(nt + 1) * 512],
                                    so[:, (nt - 1) * 512:(nt + 1) * 512])
    return nc


def _split8(x, s):
    xs = np.asarray(x, np.float32) * np.float32(s)
    hi = xs.astype(E4M3)
    lo = (xs - hi.astype(np.float32)).astype(E4M3)
    return hi, lo


def _pack_w_qk(W):
    """[2048in, 2048out] -> [16 ob, 128 p, 16 g, 2 (lo,hi), 128 col] fp8."""
    hi, lo = _split8(W, SW)
    out = np.empty((16, 128, 16, 2, 128), E4M3)
    hi4 = hi.reshape(16, 128, 16, 128)   # [g, p, ob, col]
    lo4 = lo.reshape(16, 128, 16, 128)
    out[:, :, :, 0] = lo4.transpose(2, 1, 0, 3)
    out[:, :, :, 1] = hi4.transpose(2, 1, 0, 3)
    return np.ascontiguousarray(out)


def _pack_w_v(W):
    """[2048in, 2048out] -> [4 og, 128 p, 16 g, 2 (lo,hi), 512 col] fp8."""
    hi, lo = _split8(W, SW)
    out = np.empty((4, 128, 16, 2, 512), E4M3)
    hi4 = hi.reshape(16, 128, 4, 512)    # [g, p, og, col]
    lo4 = lo.reshape(16, 128, 4, 512)
    out[:, :, :, 0] = lo4.transpose(2, 1, 0, 3)
    out[:, :, :, 1] = hi4.transpose(2, 1, 0, 3)
    return np.ascontiguousarray(out)


def _pack_w_o(W):
    """[2048in, 2048out] -> [128 p, 16 g, 2 (lo,hi), 2048 col] fp8."""
    hi, lo = _split8(W, SW)
    out = np.empty((128, 16, 2, 2048), E4M3)
    out[:, :, 0] = lo.reshape(16, 128, 2048).transpose(1, 0, 2)
    out[:, :, 1] = hi.reshape(16, 128, 2048).transpose(1, 0, 2)
    return np.ascontiguousarray(out)


def _host_inputs(hidden_states, Wq, Wk, Wv, Wo, T):
    """Build the 8 per-core input maps."""
    TH = T + WIN
    inv_freq = 1.0 / (THETA ** (np.arange(0, HD, 2, dtype=np.float32) / HD))

    qq = np.arange(WIN)[None, :]
    kk = np.arange(128)[:, None]
    tri23 = np.concatenate([(qq >= kk), (qq >= kk + 128)], 1).astype(ml_dtypes.bfloat16)
    onesm_bf = np.ones((128, 128), ml_dtypes.bfloat16)

    wq_p = _pack_w_qk(Wq)
    wk_p = _pack_w_qk(Wk)
    wv_p = _pack_w_v(Wv)
    wo_p = _pack_w_o(Wo)

    in_maps = []
    for core in range(8):
        b, sh = divmod(core, NSH)
        t0 = sh * T
        hs = np.zeros((TH, DIMS), np.float32)
        lo_ = max(0, t0 - WIN)
        hs[WIN - (t0 - lo_):] = hidden_states[b, lo_:t0 + T]
        hT = hs.T                                         # [2048, TH]
        xh, xl = _split8(hT, SX)
        xhl = np.empty((128, 16, 2, TH), E4M3)
        xhl[:, :, 0] = xh.reshape(16, 128, TH).transpose(1, 0, 2)
        xhl[:, :, 1] = xl.reshape(16, 128, TH).transpose(1, 0, 2)

        pos = np.arange(t0 - WIN, t0 + T, dtype=np.float32)
        f = np.outer(inv_freq, pos)                      # [64, TH]
        cos = np.concatenate([np.cos(f), np.cos(f)], 0)  # [128, TH]
        sin = np.sin(f)
        sins = np.concatenate([-sin, sin], 0)
        pg = np.full((128, 1), -1e30 if sh == 0 else 0.0, np.float32)
        in_maps.append({
            "XHL": np.ascontiguousarray(xhl),
            "WQHL": wq_p, "WKHL": wk_p, "WVHL": wv_p, "WOHL": wo_p,
            "COSS": cos.astype(ml_dtypes.bfloat16),
            "SINS": sins.astype(ml_dtypes.bfloat16),
            "TRI23": tri23, "PGATE": pg, "ONESM": onesm_bf,
        })
    return in_maps


_CACHE = {}


def run(hidden_states, Wq, Wk, Wv, Wo, T=S // NSH, **spmd_kwargs):
    key = T
    if key not in _CACHE:
        nc = bacc.Bacc(None)
        build(nc, T)
        nc.finalize()
        _CACHE[key] = nc
    nc = _CACHE[key]
    in_maps = _host_inputs(hidden_states, Wq, Wk, Wv, Wo, T)
    res = run_bass_kernel_spmd(nc, in_maps, core_ids=list(range(8)), **spmd_kwargs)
    outs = [res.results[i]["OUT"] for i in range(8)]
    full = np.empty((B, NSH * T, DIMS), np.float32)
    for core in range(8):
        b, sh = divmod(core, NSH)
        full[b, sh * T:(sh + 1) * T] = outs[core]
    return full, res


def kernel(hidden_states, Wq, Wk, Wv, Wo):
    out, _ = run(np.asarray(hidden_states), Wq, Wk, Wv, Wo)
    return out


# revision 46
# speedup vs baseline: 1.0228x; 1.0120x over previous
"""Block sliding-window attention on 8 TRN2 NeuronCores.

Sharding: sequence-parallel, no collectives. 8 shards = (batch b in {0,1}) x
(quarter s in 0..3); each core owns 2048 consecutive tokens of one batch plus
a 256-token K/V halo from the previous quarter (zeros + -inf gate for the
first quarter).

All four projection matmuls run as fp8-e4m3 DoubleRow (2 contraction rows per
partition, 2x PE column rate) with a 3-term residual split for bf16-level
accuracy:  x@W ~= xh@Wh + xh@Wl + xl@Wh,  where xh = fp8(s*x) and
xl = fp8(s*x - xh) share one scale per tensor, so all three terms accumulate
in a single PSUM group (cost 12 col-cycles per 2048-deep column vs 16 bf16).

Per-core pipeline:
  P1: Q = x@Wq with RoPE fused on the PSUM path (ACT scaled copy -> bf16,
      partition-swap SBUF DMA for rotate-half, 3 DVE bf16 muls/adds with
      bf16 cos / pre-signed sin), roped Q kept resident in SBUF; roped K
      stored chunk-major to DRAM; V natural to DRAM.
  P2: per 256-token chunk and head: S^T = K Q^T per 128-key block into one
      [128,1024] PSUM tile, exp on ACT (scale 1/sqrt(128), -1e30 bias gates
      the no-previous case), 0/1 triangular mask multiply on DVE,
      denominator via all-ones matmul, O^T = V^T P^T, reciprocal on DVE,
      normalize on GpSimd(Pool), store O^T token-tile-major bf16.
  P3: out = O @ Wo via the same fp8 DoubleRow split (O quantized on the fly:
      ACT scaled copy for hi, fused DVE scalar_tensor_tensor for lo).
"""
import sys

try:
    import concourse  # noqa: F401
except ImportError:
    sys.path.insert(0, '/opt/trn_rl_repo')

import ml_dtypes
import numpy as np

import concourse.bacc as bacc
import concourse.mybir as mybir
import concourse.tile as tile
from concourse.bass_utils import run_bass_kernel_spmd

f32 = mybir.dt.float32
bf16 = mybir.dt.bfloat16
f8 = mybir.dt.float8e4
AF = mybir.ActivationFunctionType
ALU = mybir.AluOpType
DR = mybir.MatmulPerfMode.DoubleRow

DIMS = 2048
HEADS = 16
HD = 128           # head dim
WIN = 256          # window / chunk
B, S = 2, 8192
NSH = 4            # seq shards per batch
THETA = 10000.0
ISQ = float(1.0 / np.sqrt(HD))
SX = 16.0          # fp8 scale for activations (x, O)
SW = 1024.0        # fp8 scale for weights
OSC = float(1.0 / (SX * SW))
E4M3 = ml_dtypes.float8_e4m3


def tok_tiles(n):
    out, a = [], 0
    while a < n:
        w = min(512, n - a)
        out.append((a, w))
        a += w
    return out


def build(nc, T, phases=(1, 2, 3)):
    """Emit the per-core program. T = local tokens (multiple of 512)."""
    TH = T + WIN                      # with halo
    NC_ = T // WIN                    # chunks
    NTB = TH // 128                   # 128-token blocks incl halo
    NTT = T // 128                    # own 128-token blocks

    XHL = nc.dram_tensor("XHL", [128, 16, 2, TH], f8, kind="ExternalInput")
    WQHL = nc.dram_tensor("WQHL", [16, 128, 16, 2, 128], f8, kind="ExternalInput")
    WKHL = nc.dram_tensor("WKHL", [16, 128, 16, 2, 128], f8, kind="ExternalInput")
    WVHL = nc.dram_tensor("WVHL", [4, 128, 16, 2, 512], f8, kind="ExternalInput")
    WOHL = nc.dram_tensor("WOHL", [128, 16, 2, DIMS], f8, kind="ExternalInput")
    COSS = nc.dram_tensor("COSS", [128, TH], bf16, kind="ExternalInput")
    SINS = nc.dram_tensor("SINS", [128, TH], bf16, kind="ExternalInput")
    TRI23 = nc.dram_tensor("TRI23", [128, 2 * WIN], bf16, kind="ExternalInput")
    PGATE = nc.dram_tensor("PGATE", [128, 1], f32, kind="ExternalInput")
    ONESM = nc.dram_tensor("ONESM", [128, 128], bf16, kind="ExternalInput")
    OUT = nc.dram_tensor("OUT", [T, DIMS], f32, kind="ExternalOutput")

    KTS = nc.dram_tensor("KTS", [128, NC_ + 1, HEADS, WIN], bf16)  # chunk-major roped K^T
    VS = nc.dram_tensor("VS", [TH, DIMS], bf16)                    # V natural
    OTS = nc.dram_tensor("OTS", [128, NTT, HEADS, 128], bf16)      # tt-major normalized O^T

    with tile.TileContext(nc) as tc:
        with tc.tile_pool(name="cst", bufs=1) as cst, \
             tc.tile_pool(name="qres", bufs=1) as qres:
            tri23 = cst.tile([128, 2 * WIN], bf16)
            pgate = cst.tile([128, 1], f32)
            onesm = cst.tile([128, 128], bf16)
            nc.scalar.dma_start(tri23[:], TRI23[:])
            nc.scalar.dma_start(pgate[:], PGATE[:])
            nc.scalar.dma_start(onesm[:], ONESM[:])
            qts = qres.tile([128, HEADS, T], bf16)   # roped Q^T, SBUF-resident

            # ---------------- P1: projections + RoPE ----------------
            if 1 in phases:
              with tc.tile_pool(name="p1x", bufs=1) as p1x, \
                 tc.tile_pool(name="trig", bufs=1) as trig, \
                 tc.tile_pool(name="wp", bufs=2) as wp, \
                 tc.tile_pool(name="wvp", bufs=2) as wvp, \
                 tc.tile_pool(name="vbp", bufs=3) as vbp, \
                 tc.tile_pool(name="rp", bufs=3) as rp, \
                 tc.tile_pool(name="pp", bufs=4, space="PSUM") as pp, \
                 tc.tile_pool(name="ppv", bufs=4, space="PSUM") as ppv:
                xhl = p1x.tile([128, 16, 2, TH], f8)
                wt0 = wp.tile([128, 16, 2, 128], f8, name="wt")
                with tc.high_priority():
                    nc.sync.dma_start(wt0[:], WQHL[0])
                for hl in (0, 1):
                    for g in range(0, 16, 2):
                        nc.sync.dma_start(xhl[:, g:g + 2, hl],
                                          XHL[:, g:g + 2, hl])
                coss = trig.tile([128, TH], bf16)
                sins = trig.tile([128, TH], bf16)
                nc.scalar.dma_start(coss[:], COSS[:])
                nc.scalar.dma_start(sins[:], SINS[:])

                # Q (toff=WIN, RoPE pos=WIN+a, dest SBUF qts) and
                # K (toff=0, RoPE pos=a, dest DRAM KTS chunk-major)
                for W_, toff, tlen, is_q in ((WQHL, WIN, T, True),
                                             (WKHL, 0, TH, False)):
                    for ob in range(HEADS):
                        if is_q and ob == 0:
                            wt = wt0
                        else:
                            wt = wp.tile([128, 16, 2, 128], f8, name="wt")
                            nc.sync.dma_start(wt[:], W_[ob])
                        for a, w in tok_tiles(tlen):
                            ps = pp.tile([128, 512], f32, name="ps")
                            tsl = slice(toff + a, toff + a + w)
                            for kg in range(8):
                                nc.tensor.matmul(
                                    ps[:, :w], wt[:, 2 * kg:2 * kg + 2, 1],
                                    xhl[:, 2 * kg:2 * kg + 2, 0, tsl],
                                    start=(kg == 0), stop=False, perf_mode=DR)
                            for g in range(16):
                                nc.tensor.matmul(
                                    ps[:, :w], wt[:, g, :],
                                    xhl[:, g, :, tsl],
                                    start=False, stop=(g == 15), perf_mode=DR)
                            raw = rp.tile([128, 512], bf16, name="raw")
                            nc.scalar.activation(raw[:, :w], ps[:, :w],
                                                 AF.Copy, scale=OSC)
                            rot = rp.tile([128, 512], bf16, name="rot")
                            nc.sync.dma_start(rot[0:64, :w], raw[64:128, :w])
                            nc.sync.dma_start(rot[64:128, :w], raw[0:64, :w])
                            pos = toff + a
                            t1 = rp.tile([128, 512], bf16, name="t1")
                            nc.vector.tensor_mul(t1[:, :w], rot[:, :w],
                                                 sins[:, pos:pos + w])
                            t2 = rp.tile([128, 512], bf16, name="t2")
                            nc.vector.tensor_mul(t2[:, :w], raw[:, :w],
                                                 coss[:, pos:pos + w])
                            if is_q:
                                nc.vector.tensor_add(
                                    qts[:, ob, a:a + w], t2[:, :w], t1[:, :w])
                            else:
                                rop = rp.tile([128, 512], bf16, name="rop")
                                nc.vector.tensor_add(rop[:, :w], t2[:, :w],
                                                     t1[:, :w])
                                c0 = a // WIN
                                nw = w // WIN
                                nc.sync.dma_start(
                                    KTS[:, c0:c0 + nw, ob], rop[:, :w])

                # V natural: lhsT = x block [128in(x2), 128tok], rhs = Wv rows
                for og in range(4):
                    wv = wvp.tile([128, 16, 2, 512], f8, name="wv")
                    nc.sync.dma_start(wv[:], WVHL[og])
                    for tb in range(NTB):
                        ps = ppv.tile([128, 512], f32, name="psv")
                        tsl = slice(tb * 128, (tb + 1) * 128)
                        for kg in range(8):
                            nc.tensor.matmul(
                                ps[:], xhl[:, 2 * kg:2 * kg + 2, 0, tsl],
                                wv[:, 2 * kg:2 * kg + 2, 1],
                                start=(kg == 0), stop=False, perf_mode=DR)
                            nc.tensor.matmul(
                                ps[:], xhl[:, 2 * kg, :, tsl],
                                wv[:, 2 * kg, :],
                                start=False, stop=False, perf_mode=DR)
                            nc.tensor.matmul(
                                ps[:], xhl[:, 2 * kg + 1, :, tsl],
                                wv[:, 2 * kg + 1, :],
                                start=False, stop=(kg == 7), perf_mode=DR)
                        vb = vbp.tile([128, 512], bf16, name="vb")
                        nc.vector.tensor_scalar_mul(vb[:], ps[:], OSC)
                        nc.sync.dma_start(
                            VS[tb * 128:(tb + 1) * 128,
                               og * 512:(og + 1) * 512], vb[:])

            # ---------------- P3 weight prefetch + P2: attention ----------------
            with tc.tile_pool(name="wo3", bufs=1) as wo3:
              wo = wo3.tile([128, 16, 2, DIMS], f8)

              if 2 in phases:
                with tc.tile_pool(name="ktp", bufs=3) as ktp, \
                   tc.tile_pool(name="qk", bufs=4) as qk, \
                   tc.tile_pool(name="pbp", bufs=3) as pbp, \
                   tc.tile_pool(name="obp", bufs=4) as obp, \
                   tc.tile_pool(name="ps_s", bufs=3, space="PSUM") as ps_s, \
                   tc.tile_pool(name="ps_dpo", bufs=2, space="PSUM") as ps_dpo:
                    def kload(c):
                        kt = ktp.tile([128, HEADS, WIN], bf16, name="kt")
                        nc.sync.dma_start(kt[:], KTS[:, c])
                        return kt

                    def vload(c):
                        v = qk.tile([128, 2, DIMS], bf16, name="v")
                        nc.sync.dma_start(
                            v[:], VS[c * WIN:(c + 1) * WIN]
                            .rearrange("(tb p) c -> p tb c", p=128))
                        return v

                    kt_t = [kload(0), kload(1)]
                    v_t = [vload(0), vload(1)]

                    # S^T layout [keys, queries], 896 cols per head:
                    # kb0 [0:256], kb1 [256:512], kb2 [512:768] (left 128
                    # triangular), kb3-right [768:896] (triangular; the
                    # fully-masked kb3 x q<128 quadrant is never computed).
                    NSC = 3 * WIN + 128
                    def emit_scores(c, h, kts):
                        ps = ps_s.tile([128, NSC], f32, name="ps")
                        for kb in range(3):
                            nc.tensor.matmul(
                                ps[:, kb * WIN:(kb + 1) * WIN],
                                kts[kb // 2][:, h, (kb % 2) * 128:(kb % 2) * 128 + 128],
                                qts[:, h, c * WIN:(c + 1) * WIN],
                                start=True, stop=True)
                        nc.tensor.matmul(
                            ps[:, 3 * WIN:], kts[1][:, h, 128:256],
                            qts[:, h, c * WIN + 128:(c + 1) * WIN],
                            start=True, stop=True)
                        return ps

                    def emit_tail(c, h, ps, vs):
                        pb = pbp.tile([128, NSC], bf16, name="pb")
                        if c == 0:
                            nc.scalar.activation(pb[:, 0:2 * WIN], ps[:, 0:2 * WIN],
                                                 AF.Exp, bias=pgate[:], scale=ISQ)
                            nc.scalar.activation(pb[:, 2 * WIN:], ps[:, 2 * WIN:],
                                                 AF.Exp, scale=ISQ)
                        else:
                            nc.scalar.activation(pb[:], ps[:], AF.Exp, scale=ISQ)
                        tri1 = tri23[:, 0:128]
                        nc.gpsimd.tensor_mul(pb[:, 512:640], pb[:, 512:640], tri1)
                        nc.gpsimd.tensor_mul(pb[:, 768:896], pb[:, 768:896], tri1)
                        pdo = ps_dpo.tile([128, 2 * WIN], f32, name="pdo")
                        pd = pdo[:, 0:WIN]
                        po = pdo[:, WIN:2 * WIN]
                        for kb in range(3):
                            nc.tensor.matmul(
                                pd, onesm[:], pb[:, kb * WIN:(kb + 1) * WIN],
                                start=(kb == 0), stop=False)
                        nc.tensor.matmul(
                            pdo[:, 128:WIN], onesm[:], pb[:, 3 * WIN:],
                            start=False, stop=True, skip_group_check=True)
                        for kb in range(3):
                            nc.tensor.matmul(
                                po, vs[kb // 2][:, kb % 2, h * 128:(h + 1) * 128],
                                pb[:, kb * WIN:(kb + 1) * WIN],
                                start=(kb == 0), stop=False)
                        nc.tensor.matmul(
                            pdo[:, WIN + 128:2 * WIN],
                            vs[1][:, 1, h * 128:(h + 1) * 128],
                            pb[:, 3 * WIN:],
                            start=False, stop=True, skip_group_check=True)
                        rb = obp.tile([128, WIN], f32, name="rb")
                        with nc.allow_low_precision("softmax denominator"):
                            nc.vector.reciprocal(rb[:], pd)
                        if h % 2 == 0:
                            ot2 = obp.tile([128, 2, 2, 128], bf16, name="ot2")
                            emit_tail.ot2 = ot2
                        else:
                            ot2 = emit_tail.ot2
                        nc.vector.tensor_mul(ot2[:, :, h % 2], po, rb[:])
                        if h % 2 == 1:
                            nc.sync.dma_start(
                                OTS[:, 2 * c:2 * c + 2, h - 1:h + 1], ot2[:])

                    pending = []
                    for c in range(NC_):
                        if c + 2 <= NC_:
                            kt_t.append(kload(c + 2))
                            v_t.append(vload(c + 2))
                        if c >= 1:
                            per = -(-16 // max(1, NC_ - 1))
                            for g in range(per * (c - 1),
                                           min(16, per * c)):
                                nc.sync.dma_start(wo[:, g], WOHL[:, g])
                        kts = (kt_t[c], kt_t[c + 1])
                        vs = (v_t[c], v_t[c + 1])
                        for h in range(HEADS):
                            ps = emit_scores(c, h, kts)
                            pending.append((c, h, ps, vs))
                            if len(pending) > 2:
                                emit_tail(*pending.pop(0))
                    for ent in pending:
                        emit_tail(*ent)

              # ---------------- P3: output projection (fp8 DR) ----------------
              if 3 in phases:
                with tc.tile_pool(name="otp", bufs=3) as otp, \
                   tc.tile_pool(name="so3", bufs=3) as so3, \
                   tc.tile_pool(name="pp3", bufs=4, space="PSUM") as pp3:
                    for tt in range(NTT):
                        ots = otp.tile([128, HEADS, 128], bf16, name="ots")
                        nc.sync.dma_start(ots[:], OTS[:, tt])
                        hilo = otp.tile([128, HEADS, 2, 128], f8, name="hilo")
                        nc.scalar.activation(hilo[:, :, 0], ots[:],
                                             AF.Copy, scale=SX)
                        nc.vector.scalar_tensor_tensor(
                            hilo[:, :, 1], ots[:], SX, hilo[:, :, 0],
                            ALU.mult, ALU.subtract)
                        so = so3.tile([128, DIMS], f32, name="so")
                        for nt in range(4):
                            ps = pp3.tile([128, 512], f32, name="pp3")
                            nsl = slice(nt * 512, (nt + 1) * 512)
                            for kg in range(8):
                                nc.tensor.matmul(
                                    ps[:], hilo[:, 2 * kg:2 * kg + 2, 0],
                                    wo[:, 2 * kg:2 * kg + 2, 1, nsl],
                                    start=(kg == 0), stop=False, perf_mode=DR)
                                nc.tensor.matmul(
                                    ps[:], hilo[:, 2 * kg, :],
                                    wo[:, 2 * kg, :, nsl],
                                    start=False, stop=False, perf_mode=DR)
                                nc.tensor.matmul(
                                    ps[:], hilo[:, 2 * kg + 1, :],
                                    wo[:, 2 * kg + 1, :, nsl],
                                    start=False, stop=(kg == 7), perf_mode=DR)
                            nc.scalar.activation(so[:, nsl], ps[:], AF.Copy,
                                                 scale=OSC)
                            if nt % 2 == 1:
                                nc.sync.dma_start(
                                    OUT[tt * 128:(tt + 1) * 128,
                                        (nt - 1) * 512:(nt + 1) * 512],
                                    so[:, (nt - 1) * 512:(nt + 1) * 512])
    return nc


def _split8(x, s):
    xs = np.asarray(x, np.float32) * np.float32(s)
    hi = xs.astype(E4M3)
    lo = (xs - hi.astype(np.float32)).astype(E4M3)
    return hi, lo


def _pack_w_qk(W):
    """[2048in, 2048out] -> [16 ob, 128 p, 16 g, 2 (lo,hi), 128 col] fp8."""
    hi, lo = _split8(W, SW)
    out = np.empty((16, 128, 16, 2, 128), E4M3)
    hi4 = hi.reshape(16, 128, 16, 128)   # [g, p, ob, col]
    lo4 = lo.reshape(16, 128, 16, 128)
    out[:, :, :, 0] = lo4.transpose(2, 1, 0, 3)
    out[:, :, :, 1] = hi4.transpose(2, 1, 0, 3)
    return np.ascontiguousarray(out)


def _pack_w_v(W):
    """[2048in, 2048out] -> [4 og, 128 p, 16 g, 2 (lo,hi), 512 col] fp8."""
    hi, lo = _split8(W, SW)
    out = np.empty((4, 128, 16, 2, 512), E4M3)
    hi4 = hi.reshape(16, 128, 4, 512)    # [g, p, og, col]
    lo4 = lo.reshape(16, 128, 4, 512)
    out[:, :, :, 0] = lo4.transpose(2, 1, 0, 3)
    out[:, :, :, 1] = hi4.transpose(2, 1, 0, 3)
    return np.ascontiguousarray(out)


def _pack_w_o(W):
    """[2048in, 2048out] -> [128 p, 16 g, 2 (lo,hi), 2048 col] fp8."""
    hi, lo = _split8(W, SW)
    out = np.empty((128, 16, 2, 2048), E4M3)
    out[:, :, 0] = lo.reshape(16, 128, 2048).transpose(1, 0, 2)
    out[:, :, 1] = hi.reshape(16, 128, 2048).transpose(1, 0, 2)
    return np.ascontiguousarray(out)


def _host_inputs(hidden_states, Wq, Wk, Wv, Wo, T):
    """Build the 8 per-core input maps."""
    TH = T + WIN
    inv_freq = 1.0 / (THETA ** (np.arange(0, HD, 2, dtype=np.float32) / HD))

    qq = np.arange(WIN)[None, :]
    kk = np.arange(128)[:, None]
    tri23 = np.concatenate([(qq >= kk), (qq >= kk + 128)], 1).astype(ml_dtypes.bfloat16)
    onesm_bf = np.ones((128, 128), ml_dtypes.bfloat16)

    wq_p = _pack_w_qk(Wq)
    wk_p = _pack_w_qk(Wk)
    wv_p = _pack_w_v(Wv)
    wo_p = _pack_w_o(Wo)

    in_maps = []
    for core in range(8):
        b, sh = divmod(core, NSH)
        t0 = sh * T
        hs = np.zeros((TH, DIMS), np.float32)
        lo_ = max(0, t0 - WIN)
        hs[WIN - (t0 - lo_):] = hidden_states[b, lo_:t0 + T]
        hT = hs.T                                         # [2048, TH]
        xh, xl = _split8(hT, SX)
        xhl = np.empty((128, 16, 2, TH), E4M3)
        xhl[:, :, 0] = xh.reshape(16, 128, TH).transpose(1, 0, 2)
        xhl[:, :, 1] = xl.reshape(16, 128, TH).transpose(1, 0, 2)

        pos = np.arange(t0 - WIN, t0 + T, dtype=np.float32)
        f = np.outer(inv_freq, pos)                      # [64, TH]
        cos = np.concatenate([np.cos(f), np.cos(f)], 0)  # [128, TH]
        sin = np.sin(f)
        sins = np.concatenate([-sin, sin], 0)
        pg = np.full((128, 1), -1e30 if sh == 0 else 0.0, np.float32)
        in_maps.append({
            "XHL": np.ascontiguousarray(xhl),
            "WQHL": wq_p, "WKHL": wk_p, "WVHL": wv_p, "WOHL": wo_p,
            "COSS": cos.astype(ml_dtypes.bfloat16),
            "SINS": sins.astype(ml_dtypes.bfloat16),
            "TRI23": tri23, "PGATE": pg, "ONESM": onesm_bf,
        })
    return in_maps


_CACHE = {}


def run(hidden_states, Wq, Wk, Wv, Wo, T=S // NSH, **spmd_kwargs):
    key = T
    if key not in _CACHE:
        nc = bacc.Bacc(None)
        build(nc, T)
        nc.finalize()
        _CACHE[key] = nc
    nc = _CACHE[key]
    in_maps = _host_inputs(hidden_states, Wq, Wk, Wv, Wo, T)
    res = run_bass_kernel_spmd(nc, in_maps, core_ids=list(range(8)), **spmd_kwargs)
    outs = [res.results[i]["OUT"] for i in range(8)]
    full = np.empty((B, NSH * T, DIMS), np.float32)
    for core in range(8):
        b, sh = divmod(core, NSH)
        full[b, sh * T:(sh + 1) * T] = outs[core]
    return full, res


def kernel(hidden_states, Wq, Wk, Wv, Wo):
    out, _ = run(np.asarray(hidden_states), Wq, Wk, Wv, Wo)
    return out
